# revision 16
# baseline (speedup 1.0000x reference)
"""Trainium2 Bass kernel for nn_ApproxAct (piecewise-linear activation, 255 hinges).

out[i] = sum_k w_k * relu(x[i] - b_k),  w/b derived from (x_list, y_list) knot
tables on the host (257-entry prep, O(K) work).  The 1M-element hinge
evaluation runs on 8 NeuronCores, data-parallel over rows of x.

Per-core strategy: all 255 hinges evaluated exactly in fp32, split across
three concurrent engine lanes (knot values baked at build time; the kernel
compiles per call, after seeing the inputs):
  AD: ACT relu(x + bias_k) -> DVE stt mac into a PSUM accumulator
      (PSUM keeps DVE off the DVE/GpSimd shared SBUF port)
  AG: ACT relu           -> GpSimd stt mac into an SBUF accumulator
  GD: GpSimd tensor_scalar relu (1-input, line rate) -> DVE stt mac
"""

import numpy as np

M_TOTAL = 1_000_000
N_CORES = 8
P = 128
F = 977  # 128*977 = 125056 per core; 8 cores cover 1000448 >= 1e6
PER_CORE = P * F
K = 255
BOUND_LO, BOUND_HI = -100.0, 100.0

# Lane sizes (sum = K): AD = ACT->DVE stt, AG = ACT(prescaled)->GP tt-add,
# DD = DVE ts-relu + DVE stt (self-contained)
N_AD = 120
N_AG = 98
N_DD = K - N_AD - N_AG


def _tables(x_list, y_list):
    """Host-side knot prep, mimicking the fp32 reference exactly."""
    x = np.sort(np.clip(x_list.astype(np.float32), BOUND_LO, BOUND_HI))
    x[0] = np.float32(BOUND_LO * 2)
    x[-1] = np.float32(BOUND_HI * 2)
    y = y_list.astype(np.float32).copy()
    y[0] = 0.0
    y[1] = 0.0
    y[-2] = x[-2]
    y[-1] = x[-1]
    slope = (np.diff(y) / (np.diff(x) + np.float32(1e-8))).astype(np.float32)
    w = np.diff(slope).astype(np.float32)
    b = x[1:-1].astype(np.float32)
    return w, b


def _build_graph(w, b, repeat=1):
    import concourse.bacc as bacc
    import concourse.mybir as mybir
    from concourse.tile import TileContext

    f32 = mybir.dt.float32
    mult = mybir.AluOpType.mult
    add = mybir.AluOpType.add
    sub = mybir.AluOpType.subtract
    mx = mybir.AluOpType.max

    nc = bacc.Bacc(None, target_bir_lowering=False)
    x_in = nc.declare_dram_parameter("xin", [P, F + 2 * K], f32, isOutput=False)
    out_d = nc.declare_dram_parameter("out", [P, F], f32, isOutput=True)

    # interleaved emission order: spread lanes so every engine has early work
    counters = {"AD": 0, "AG": N_AD, "DD": N_AD + N_AG}
    seq = []
    remaining = {"AD": N_AD, "AG": N_AG, "DD": N_DD}
    total = K
    while total > 0:
        for lane in ("AD", "DD", "AG"):
            if remaining[lane] > 0:
                seq.append((lane, counters[lane]))
                counters[lane] += 1
                remaining[lane] -= 1
                total -= 1

    with TileContext(nc) as tc:
        with (
            tc.tile_pool(name="io", bufs=1) as io_pool,
            tc.tile_pool(name="psum", bufs=1, space="PSUM") as psum_pool,
            tc.tile_pool(name="rp", bufs=4) as rp,
        ):
            xin_t = io_pool.tile([P, F + 2 * K], f32)
            xt = xin_t[:, :F]
            nbt = xin_t[:, F:F + K]          # -b_k columns
            sbt = xin_t[:, F + K:]           # -|w_k|*b_k columns
            acc_d = psum_pool.tile([P, F], f32)
            acc_gp = io_pool.tile([P, F], f32)
            acc_gn = io_pool.tile([P, F], f32)
            res = io_pool.tile([P, F], f32)

            nc.sync.dma_start(out=xin_t[:], in_=x_in[:])

            for _ in range(repeat):
                first_d, first_gp, first_gn = [True], [True], [True]

                for lane, k in seq:
                    if lane == "AD":
                        r = rp.tile([P, F], f32, name="r_ad", tag="r_ad")
                        nc.scalar.activation(
                            r[:], xt, mybir.ActivationFunctionType.Relu,
                            bias=nbt[:, k:k + 1], scale=1.0,
                        )
                        if first_d[0]:
                            nc.vector.tensor_scalar_mul(acc_d[:], r[:], float(w[k]))
                            first_d[0] = False
                        else:
                            nc.vector.scalar_tensor_tensor(
                                out=acc_d[:], in0=r[:], scalar=float(w[k]),
                                in1=acc_d[:], op0=mult, op1=add,
                            )
                    elif lane == "AG":
                        # r' = |w_k| * relu(x - b_k), sign handled by accumulator
                        r = rp.tile([P, F], f32, name="r_ag", tag="r_ag")
                        nc.scalar.activation(
                            r[:], xt, mybir.ActivationFunctionType.Relu,
                            bias=sbt[:, k:k + 1], scale=float(abs(w[k])),
                        )
                        acc_g, flag = (
                            (acc_gp, first_gp) if w[k] >= 0 else (acc_gn, first_gn)
                        )
                        if flag[0]:
                            nc.gpsimd.tensor_copy(out=acc_g[:], in_=r[:])
                            flag[0] = False
                        else:
                            nc.gpsimd.tensor_tensor(
                                out=acc_g[:], in0=acc_g[:], in1=r[:], op=add,
                            )
                    else:  # DD: DVE ts-relu + DVE stt mac
                        r = rp.tile([P, F], f32, name="r_dd", tag="r_dd")
                        nc.vector.tensor_scalar(
                            r[:], xt, float(b[k]), 0.0, sub, mx,
                        )
                        nc.vector.scalar_tensor_tensor(
                            out=acc_d[:], in0=r[:], scalar=float(w[k]),
                            in1=acc_d[:], op0=mult, op1=add,
                        )

                # res = (acc_d + acc_gp) - acc_gn  (DVE; GP is done by now)
                cur = acc_d
                if not first_gp[0]:
                    nc.vector.tensor_tensor(
                        out=res[:], in0=cur[:], in1=acc_gp[:], op=add,
                    )
                    cur = res
                if not first_gn[0]:
                    nc.vector.tensor_tensor(
                        out=res[:], in0=cur[:], in1=acc_gn[:], op=sub,
                    )
                    cur = res
                if cur is not res:
                    nc.vector.tensor_copy(out=res[:], in_=cur[:])
            nc.sync.dma_start(out=out_d[:], in_=res[:])
    return nc


def _prep_inputs(x, x_list, y_list):
    w, b = _tables(np.asarray(x_list), np.asarray(y_list))
    x_flat = np.ascontiguousarray(np.asarray(x, dtype=np.float32).reshape(-1))
    assert x_flat.size == M_TOTAL, x_flat.size
    padded = np.zeros(N_CORES * PER_CORE, np.float32)
    padded[:M_TOTAL] = x_flat
    shards = padded.reshape(N_CORES, P, F)
    nb_tile = np.broadcast_to((-b).reshape(1, K), (P, K)).astype(np.float32)
    sb = (-(np.abs(w.astype(np.float64)) * b.astype(np.float64))).astype(np.float32)
    sb_tile = np.broadcast_to(sb.reshape(1, K), (P, K)).astype(np.float32)
    in_maps = []
    for i in range(N_CORES):
        xin = np.concatenate([shards[i], nb_tile, sb_tile], axis=1)
        in_maps.append({"xin": np.ascontiguousarray(xin)})
    return w, b, in_maps


def run(x, x_list, y_list, trace=False, repeat=1, **spmd_kwargs):
    from concourse.bass_utils import run_bass_kernel_spmd

    w, b, in_maps = _prep_inputs(x, x_list, y_list)
    nc = _build_graph(w, b, repeat=repeat)
    if not nc.is_finalized():
        nc.finalize()
    res = run_bass_kernel_spmd(
        nc, in_maps, core_ids=list(range(N_CORES)), trace=trace, **spmd_kwargs
    )
    outs = np.stack([res.results[i]["out"] for i in range(N_CORES)])
    full = outs.reshape(-1)[:M_TOTAL].reshape(M_TOTAL, 1).astype(np.float32)
    return full, res


def kernel(x, x_list, y_list):
    full, _ = run(x, x_list, y_list, trace=False)
    return full


# revision 21
# speedup vs baseline: 1.1127x; 1.1127x over previous
"""Trainium2 Bass kernel for nn_ApproxAct (piecewise-linear activation, 255 hinges).

out[i] = sum_k w_k * relu(x[i] - b_k),  w/b derived from (x_list, y_list) knot
tables on the host (257-entry prep, O(K) work).  The 1M-element hinge
evaluation runs on 8 NeuronCores, data-parallel over rows of x.

Per-core strategy: all 255 hinges evaluated exactly in fp32, split across
three concurrent engine lanes (knot values baked at build time; the kernel
compiles per call, after seeing the inputs):
  AD: ACT relu(x + bias_k) -> DVE stt mac into a PSUM accumulator
      (PSUM keeps DVE off the DVE/GpSimd shared SBUF port)
  AG: ACT relu           -> GpSimd stt mac into an SBUF accumulator
  GD: GpSimd tensor_scalar relu (1-input, line rate) -> DVE stt mac
"""

import numpy as np

M_TOTAL = 1_000_000
N_CORES = 8
P = 128
F = 977  # 128*977 = 125056 per core; 8 cores cover 1000448 >= 1e6
PER_CORE = P * F
K = 255
BOUND_LO, BOUND_HI = -100.0, 100.0

# Lane sizes (sum = K):
#   AD = ACT relu -> DVE stt mac (PSUM accumulator)
#   AG = ACT prescaled relu -> GpSimd tt-add (pos/neg SBUF accumulators)
#   AC = ACT prescaled relu -> GpSimd CCE accumulate-DMA (2x pos + 2x neg accs)
#   DD = DVE ts-relu + DVE stt (self-contained)
SPLITS = (71, 72, 50, 62)  # (N_AD, N_AG, N_AC, N_DD), sums to K


def _tables(x_list, y_list):
    """Host-side knot prep, mimicking the fp32 reference exactly."""
    x = np.sort(np.clip(x_list.astype(np.float32), BOUND_LO, BOUND_HI))
    x[0] = np.float32(BOUND_LO * 2)
    x[-1] = np.float32(BOUND_HI * 2)
    y = y_list.astype(np.float32).copy()
    y[0] = 0.0
    y[1] = 0.0
    y[-2] = x[-2]
    y[-1] = x[-1]
    slope = (np.diff(y) / (np.diff(x) + np.float32(1e-8))).astype(np.float32)
    w = np.diff(slope).astype(np.float32)
    b = x[1:-1].astype(np.float32)
    return w, b


def _build_graph(w, b, repeat=1, splits=None):
    import concourse.bacc as bacc
    import concourse.mybir as mybir
    from concourse.tile import TileContext

    f32 = mybir.dt.float32
    mult = mybir.AluOpType.mult
    add = mybir.AluOpType.add
    sub = mybir.AluOpType.subtract
    mx = mybir.AluOpType.max

    n_ad, n_ag, n_ac, n_dd = splits or SPLITS
    assert n_ad + n_ag + n_ac + n_dd == K

    nc = bacc.Bacc(None, target_bir_lowering=False)
    x_in = nc.declare_dram_parameter("xin", [P, F + 2 * K], f32, isOutput=False)
    out_d = nc.declare_dram_parameter("out", [P, F], f32, isOutput=True)

    # interleaved emission order: spread lanes so every engine has early work
    counters = {"AD": 0, "AG": n_ad, "AC": n_ad + n_ag, "DD": n_ad + n_ag + n_ac}
    seq = []
    remaining = {"AD": n_ad, "AG": n_ag, "AC": n_ac, "DD": n_dd}
    total = K
    while total > 0:
        for lane in ("AD", "DD", "AC", "AG"):
            if remaining[lane] > 0:
                seq.append((lane, counters[lane]))
                counters[lane] += 1
                remaining[lane] -= 1
                total -= 1

    with TileContext(nc) as tc:
        with (
            tc.tile_pool(name="io", bufs=1) as io_pool,
            tc.tile_pool(name="psum", bufs=1, space="PSUM") as psum_pool,
            tc.tile_pool(name="rp", bufs=4) as rp,
        ):
            xin_t = io_pool.tile([P, F + 2 * K], f32)
            xt = xin_t[:, :F]
            nbt = xin_t[:, F:F + K]          # -b_k columns
            sbt = xin_t[:, F + K:]           # -|w_k|*b_k columns
            acc_d = psum_pool.tile([P, F], f32)
            acc_gp = io_pool.tile([P, F], f32)
            acc_gn = io_pool.tile([P, F], f32)
            acc_c = [
                io_pool.tile([P, F], f32, name=f"acc_c{i}") for i in range(4)
            ]  # CCE accumulators: [pos0, pos1, neg0, neg1]
            res = io_pool.tile([P, F], f32)

            nc.sync.dma_start(out=xin_t[:], in_=x_in[:])

            for _ in range(repeat):
                first_d, first_gp, first_gn = [True], [True], [True]
                used_c = [False] * 4
                if n_ac:
                    for t in acc_c:
                        nc.gpsimd.memset(t[:], 0.0)
                ccnt = [0]

                for lane, k in seq:
                    if lane == "AD":
                        r = rp.tile([P, F], f32, name="r_ad", tag="r_ad")
                        nc.scalar.activation(
                            r[:], xt, mybir.ActivationFunctionType.Relu,
                            bias=nbt[:, k:k + 1], scale=1.0,
                        )
                        if first_d[0]:
                            nc.vector.tensor_scalar_mul(acc_d[:], r[:], float(w[k]))
                            first_d[0] = False
                        else:
                            nc.vector.scalar_tensor_tensor(
                                out=acc_d[:], in0=r[:], scalar=float(w[k]),
                                in1=acc_d[:], op0=mult, op1=add,
                            )
                    elif lane == "AG":
                        # r' = |w_k| * relu(x - b_k), sign handled by accumulator
                        r = rp.tile([P, F], f32, name="r_ag", tag="r_ag")
                        nc.scalar.activation(
                            r[:], xt, mybir.ActivationFunctionType.Relu,
                            bias=sbt[:, k:k + 1], scale=float(abs(w[k])),
                        )
                        acc_g, flag = (
                            (acc_gp, first_gp) if w[k] >= 0 else (acc_gn, first_gn)
                        )
                        if flag[0]:
                            nc.gpsimd.tensor_copy(out=acc_g[:], in_=r[:])
                            flag[0] = False
                        else:
                            nc.gpsimd.tensor_tensor(
                                out=acc_g[:], in0=acc_g[:], in1=r[:], op=add,
                            )
                    elif lane == "AC":
                        # r' = |w_k| * relu(x - b_k) accumulated by compute-DMA
                        r = rp.tile([P, F], f32, name="r_ac", tag="r_ac")
                        nc.scalar.activation(
                            r[:], xt, mybir.ActivationFunctionType.Relu,
                            bias=sbt[:, k:k + 1], scale=float(abs(w[k])),
                        )
                        par = ccnt[0] % 2
                        ccnt[0] += 1
                        ci = par if w[k] >= 0 else 2 + par
                        used_c[ci] = True
                        nc.gpsimd.dma_start(
                            out=acc_c[ci][:], in_=r[:], accum_op=add,
                        )
                    else:  # DD: DVE ts-relu + DVE stt mac
                        r = rp.tile([P, F], f32, name="r_dd", tag="r_dd")
                        nc.vector.tensor_scalar(
                            r[:], xt, float(b[k]), 0.0, sub, mx,
                        )
                        nc.vector.scalar_tensor_tensor(
                            out=acc_d[:], in0=r[:], scalar=float(w[k]),
                            in1=acc_d[:], op0=mult, op1=add,
                        )

                # res = acc_d + acc_gp - acc_gn + cce_pos - cce_neg  (DVE)
                cur = acc_d
                terms = []
                if not first_gp[0]:
                    terms.append((acc_gp, add))
                if not first_gn[0]:
                    terms.append((acc_gn, sub))
                for i, t in enumerate(acc_c):
                    if used_c[i]:
                        terms.append((t, add if i < 2 else sub))
                for t, op in terms:
                    nc.vector.tensor_tensor(out=res[:], in0=cur[:], in1=t[:], op=op)
                    cur = res
                if cur is not res:
                    nc.vector.tensor_copy(out=res[:], in_=cur[:])
            nc.sync.dma_start(out=out_d[:], in_=res[:])
    return nc


def _prep_inputs(x, x_list, y_list):
    w, b = _tables(np.asarray(x_list), np.asarray(y_list))
    x_flat = np.ascontiguousarray(np.asarray(x, dtype=np.float32).reshape(-1))
    assert x_flat.size == M_TOTAL, x_flat.size
    padded = np.zeros(N_CORES * PER_CORE, np.float32)
    padded[:M_TOTAL] = x_flat
    shards = padded.reshape(N_CORES, P, F)
    nb_tile = np.broadcast_to((-b).reshape(1, K), (P, K)).astype(np.float32)
    sb = (-(np.abs(w.astype(np.float64)) * b.astype(np.float64))).astype(np.float32)
    sb_tile = np.broadcast_to(sb.reshape(1, K), (P, K)).astype(np.float32)
    in_maps = []
    for i in range(N_CORES):
        xin = np.concatenate([shards[i], nb_tile, sb_tile], axis=1)
        in_maps.append({"xin": np.ascontiguousarray(xin)})
    return w, b, in_maps


def run(x, x_list, y_list, trace=False, repeat=1, **spmd_kwargs):
    from concourse.bass_utils import run_bass_kernel_spmd

    w, b, in_maps = _prep_inputs(x, x_list, y_list)
    nc = _build_graph(w, b, repeat=repeat)
    if not nc.is_finalized():
        nc.finalize()
    res = run_bass_kernel_spmd(
        nc, in_maps, core_ids=list(range(N_CORES)), trace=trace, **spmd_kwargs
    )
    outs = np.stack([res.results[i]["out"] for i in range(N_CORES)])
    full = outs.reshape(-1)[:M_TOTAL].reshape(M_TOTAL, 1).astype(np.float32)
    return full, res


def kernel(x, x_list, y_list):
    full, _ = run(x, x_list, y_list, trace=False)
    return full


# revision 33
# speedup vs baseline: 1.1302x; 1.0157x over previous
"""Trainium2 Bass kernel for nn_ApproxAct (piecewise-linear activation, 255 hinges).

out[i] = sum_k w_k * relu(x[i] - b_k),  w/b derived from (x_list, y_list) knot
tables on the host (257-entry prep, O(K) work).  The 1M-element hinge
evaluation runs on 8 NeuronCores, data-parallel over rows of x.

Per-core strategy: all 255 hinges evaluated exactly in fp32, split across
four concurrent engine lanes (knot values baked at build time; the kernel
compiles per call, after seeing the inputs):
  AD: ACT relu(x + bias_k) -> VectorE fused mul-add into a PSUM accumulator
      (PSUM keeps VectorE off the VectorE/GpSimd shared SBUF port)
  AG: ACT prescaled relu -> GpSimd tensor_tensor adds (pos/neg accumulators)
  AC: ACT prescaled relu -> compute-DMA (SWDGE accum_op=add) accumulators
  DD: VectorE tensor_scalar relu (fp32 2x mode) + VectorE fused mul-add
GpSimd pre-combines its accumulator pairs while VectorE drains, and the ACT
table set is pre-warmed under the input DMA.
"""

import numpy as np

M_TOTAL = 1_000_000
N_CORES = 8
P = 128
F = 977  # 128*977 = 125056 per core; 8 cores cover 1000448 >= 1e6
PER_CORE = P * F
K = 255
BOUND_LO, BOUND_HI = -100.0, 100.0

# Lane sizes (sum = K):
#   AD = ACT relu -> DVE stt mac (PSUM accumulator)
#   AG = ACT prescaled relu -> GpSimd tt-add (pos/neg SBUF accumulators)
#   AC = ACT prescaled relu -> GpSimd CCE accumulate-DMA (2x pos + 2x neg accs)
#   DD = DVE ts-relu + DVE stt (self-contained)
SPLITS = (71, 72, 50, 62)  # (N_AD, N_AG, N_AC, N_DD), sums to K


def _tables(x_list, y_list):
    """Host-side knot prep, mimicking the fp32 reference exactly."""
    x = np.sort(np.clip(x_list.astype(np.float32), BOUND_LO, BOUND_HI))
    x[0] = np.float32(BOUND_LO * 2)
    x[-1] = np.float32(BOUND_HI * 2)
    y = y_list.astype(np.float32).copy()
    y[0] = 0.0
    y[1] = 0.0
    y[-2] = x[-2]
    y[-1] = x[-1]
    slope = (np.diff(y) / (np.diff(x) + np.float32(1e-8))).astype(np.float32)
    w = np.diff(slope).astype(np.float32)
    b = x[1:-1].astype(np.float32)
    return w, b


def _build_graph(w, b, repeat=1, splits=None):
    import concourse.bacc as bacc
    import concourse.mybir as mybir
    from concourse.tile import TileContext

    f32 = mybir.dt.float32
    mult = mybir.AluOpType.mult
    add = mybir.AluOpType.add
    sub = mybir.AluOpType.subtract
    mx = mybir.AluOpType.max

    n_ad, n_ag, n_ac, n_dd = splits or SPLITS
    assert n_ad + n_ag + n_ac + n_dd == K

    nc = bacc.Bacc(None, target_bir_lowering=False)
    x_in = nc.declare_dram_parameter("xin", [P, F + 2 * K], f32, isOutput=False)
    out_d = nc.declare_dram_parameter("out", [P, F], f32, isOutput=True)

    # interleaved emission order: spread lanes so every engine has early work
    counters = {"AD": 0, "AG": n_ad, "AC": n_ad + n_ag, "DD": n_ad + n_ag + n_ac}
    seq = []
    remaining = {"AD": n_ad, "AG": n_ag, "AC": n_ac, "DD": n_dd}
    total = K
    while total > 0:
        for lane in ("AD", "DD", "AC", "AG"):
            if remaining[lane] > 0:
                seq.append((lane, counters[lane]))
                counters[lane] += 1
                remaining[lane] -= 1
                total -= 1

    with TileContext(nc) as tc:
        with (
            tc.tile_pool(name="io", bufs=1) as io_pool,
            tc.tile_pool(name="psum", bufs=1, space="PSUM") as psum_pool,
            tc.tile_pool(name="rp", bufs=4) as rp,
        ):
            xin_t = io_pool.tile([P, F + 2 * K], f32)
            xt = xin_t[:, :F]
            nbt = xin_t[:, F:F + K]          # -b_k columns
            sbt = xin_t[:, F + K:]           # -|w_k|*b_k columns
            acc_d = psum_pool.tile([P, F], f32)
            acc_gp = io_pool.tile([P, F], f32)
            acc_gn = io_pool.tile([P, F], f32)
            acc_c = [
                io_pool.tile([P, F], f32, name=f"acc_c{i}") for i in range(4)
            ]  # CCE accumulators: [pos0, pos1, neg0, neg1]
            res = io_pool.tile([P, F], f32)

            # Pre-warm the ACT table set (Relu) while the input DMA is in
            # flight: the table load (~2.7us) is data-independent.
            warm = io_pool.tile([P, 1], f32, name="warm")
            nc.vector.memset(warm[:], 0.0)
            nc.scalar.activation(
                warm[:], warm[:], mybir.ActivationFunctionType.Relu,
                bias=0.0, scale=1.0,
            )

            nc.sync.dma_start(out=xin_t[:], in_=x_in[:])

            for _ in range(repeat):
                first_d, first_gp, first_gn = [True], [True], [True]
                used_c = [False] * 4
                if n_ac:
                    for t in acc_c:
                        nc.gpsimd.memset(t[:], 0.0)
                ccnt = [0]

                for lane, k in seq:
                    if lane == "AD":
                        r = rp.tile([P, F], f32, name="r_ad", tag="r_ad")
                        nc.scalar.activation(
                            r[:], xt, mybir.ActivationFunctionType.Relu,
                            bias=nbt[:, k:k + 1], scale=1.0,
                        )
                        if first_d[0]:
                            nc.vector.tensor_scalar_mul(acc_d[:], r[:], float(w[k]))
                            first_d[0] = False
                        else:
                            nc.vector.scalar_tensor_tensor(
                                out=acc_d[:], in0=r[:], scalar=float(w[k]),
                                in1=acc_d[:], op0=mult, op1=add,
                            )
                    elif lane == "AG":
                        # r' = |w_k| * relu(x - b_k), sign handled by accumulator
                        r = rp.tile([P, F], f32, name="r_ag", tag="r_ag")
                        nc.scalar.activation(
                            r[:], xt, mybir.ActivationFunctionType.Relu,
                            bias=sbt[:, k:k + 1], scale=float(abs(w[k])),
                        )
                        acc_g, flag = (
                            (acc_gp, first_gp) if w[k] >= 0 else (acc_gn, first_gn)
                        )
                        if flag[0]:
                            nc.gpsimd.tensor_copy(out=acc_g[:], in_=r[:])
                            flag[0] = False
                        else:
                            nc.gpsimd.tensor_tensor(
                                out=acc_g[:], in0=acc_g[:], in1=r[:], op=add,
                            )
                    elif lane == "AC":
                        # r' = |w_k| * relu(x - b_k) accumulated by compute-DMA
                        r = rp.tile([P, F], f32, name="r_ac", tag="r_ac")
                        nc.scalar.activation(
                            r[:], xt, mybir.ActivationFunctionType.Relu,
                            bias=sbt[:, k:k + 1], scale=float(abs(w[k])),
                        )
                        par = ccnt[0] % 2
                        ccnt[0] += 1
                        ci = par if w[k] >= 0 else 2 + par
                        used_c[ci] = True
                        nc.gpsimd.dma_start(
                            out=acc_c[ci][:], in_=r[:], accum_op=add,
                        )
                    else:  # DD: DVE ts-relu + DVE stt mac
                        r = rp.tile([P, F], f32, name="r_dd", tag="r_dd")
                        nc.vector.tensor_scalar(
                            r[:], xt, float(b[k]), 0.0, sub, mx,
                        )
                        nc.vector.scalar_tensor_tensor(
                            out=acc_d[:], in0=r[:], scalar=float(w[k]),
                            in1=acc_d[:], op0=mult, op1=add,
                        )

                # GpSimd pre-combines its accumulator pairs while DVE finishes
                # the mac chain, shortening the serial DVE tail.
                gp_used, gn_used = not first_gp[0], not first_gn[0]
                if gp_used and gn_used:
                    nc.gpsimd.tensor_tensor(
                        out=acc_gp[:], in0=acc_gp[:], in1=acc_gn[:], op=sub,
                    )
                    gn_used = False
                if used_c[0] and used_c[1]:
                    nc.gpsimd.tensor_tensor(
                        out=acc_c[0][:], in0=acc_c[0][:], in1=acc_c[1][:], op=add,
                    )
                    used_c[1] = False
                if used_c[2] and used_c[3]:
                    nc.gpsimd.tensor_tensor(
                        out=acc_c[2][:], in0=acc_c[2][:], in1=acc_c[3][:], op=add,
                    )
                    used_c[3] = False

                # res = acc_d + acc_gp(-acc_gn) + cce_pos - cce_neg  (DVE)
                cur = acc_d
                terms = []
                if gp_used:
                    terms.append((acc_gp, add))
                if gn_used:
                    terms.append((acc_gn, sub))
                for i, t in enumerate(acc_c):
                    if used_c[i]:
                        terms.append((t, add if i < 2 else sub))
                for t, op in terms:
                    nc.vector.tensor_tensor(out=res[:], in0=cur[:], in1=t[:], op=op)
                    cur = res
                if cur is not res:
                    nc.vector.tensor_copy(out=res[:], in_=cur[:])
            nc.sync.dma_start(out=out_d[:], in_=res[:])
    return nc


def _prep_inputs(x, x_list, y_list):
    w, b = _tables(np.asarray(x_list), np.asarray(y_list))
    x_flat = np.ascontiguousarray(np.asarray(x, dtype=np.float32).reshape(-1))
    assert x_flat.size == M_TOTAL, x_flat.size
    padded = np.zeros(N_CORES * PER_CORE, np.float32)
    padded[:M_TOTAL] = x_flat
    shards = padded.reshape(N_CORES, P, F)
    nb_tile = np.broadcast_to((-b).reshape(1, K), (P, K)).astype(np.float32)
    sb = (-(np.abs(w.astype(np.float64)) * b.astype(np.float64))).astype(np.float32)
    sb_tile = np.broadcast_to(sb.reshape(1, K), (P, K)).astype(np.float32)
    in_maps = []
    for i in range(N_CORES):
        xin = np.concatenate([shards[i], nb_tile, sb_tile], axis=1)
        in_maps.append({"xin": np.ascontiguousarray(xin)})
    return w, b, in_maps


def run(x, x_list, y_list, trace=False, repeat=1, **spmd_kwargs):
    from concourse.bass_utils import run_bass_kernel_spmd

    w, b, in_maps = _prep_inputs(x, x_list, y_list)
    nc = _build_graph(w, b, repeat=repeat)
    if not nc.is_finalized():
        nc.finalize()
    res = run_bass_kernel_spmd(
        nc, in_maps, core_ids=list(range(N_CORES)), trace=trace, **spmd_kwargs
    )
    outs = np.stack([res.results[i]["out"] for i in range(N_CORES)])
    full = outs.reshape(-1)[:M_TOTAL].reshape(M_TOTAL, 1).astype(np.float32)
    return full, res


def kernel(x, x_list, y_list):
    full, _ = run(x, x_list, y_list, trace=False)
    return full


# revision 34
# speedup vs baseline: 1.1330x; 1.0025x over previous
"""Trainium2 Bass kernel for nn_ApproxAct (piecewise-linear activation, 255 hinges).

out[i] = sum_k w_k * relu(x[i] - b_k),  w/b derived from (x_list, y_list) knot
tables on the host (257-entry prep, O(K) work).  The 1M-element hinge
evaluation runs on 8 NeuronCores, data-parallel over rows of x.

Per-core strategy: all 255 hinges evaluated exactly in fp32, split across
four concurrent engine lanes (knot values baked at build time; the kernel
compiles per call, after seeing the inputs):
  AD: ACT relu(x + bias_k) -> VectorE fused mul-add into a PSUM accumulator
      (PSUM keeps VectorE off the VectorE/GpSimd shared SBUF port)
  AG: ACT prescaled relu -> GpSimd tensor_tensor adds (pos/neg accumulators)
  AC: ACT prescaled relu -> compute-DMA (SWDGE accum_op=add) accumulators
  DD: VectorE tensor_scalar relu (fp32 2x mode) + VectorE fused mul-add
GpSimd pre-combines its accumulator pairs while VectorE drains, and the ACT
table set is pre-warmed under the input DMA.
"""

import numpy as np

M_TOTAL = 1_000_000
N_CORES = 8
P = 128
F = 977  # 128*977 = 125056 per core; 8 cores cover 1000448 >= 1e6
PER_CORE = P * F
K = 255
BOUND_LO, BOUND_HI = -100.0, 100.0

# Lane sizes (sum = K):
#   AD = ACT relu -> DVE stt mac (PSUM accumulator)
#   AG = ACT prescaled relu -> GpSimd tt-add (pos/neg SBUF accumulators)
#   AC = ACT prescaled relu -> GpSimd CCE accumulate-DMA (2x pos + 2x neg accs)
#   DD = DVE ts-relu + DVE stt (self-contained)
SPLITS = (71, 72, 50, 62)  # (N_AD, N_AG, N_AC, N_DD), sums to K


def _tables(x_list, y_list):
    """Host-side knot prep, mimicking the fp32 reference exactly."""
    x = np.sort(np.clip(x_list.astype(np.float32), BOUND_LO, BOUND_HI))
    x[0] = np.float32(BOUND_LO * 2)
    x[-1] = np.float32(BOUND_HI * 2)
    y = y_list.astype(np.float32).copy()
    y[0] = 0.0
    y[1] = 0.0
    y[-2] = x[-2]
    y[-1] = x[-1]
    slope = (np.diff(y) / (np.diff(x) + np.float32(1e-8))).astype(np.float32)
    w = np.diff(slope).astype(np.float32)
    b = x[1:-1].astype(np.float32)
    return w, b


def _build_graph(w, b, repeat=1, splits=None):
    import concourse.bacc as bacc
    import concourse.mybir as mybir
    from concourse.tile import TileContext

    f32 = mybir.dt.float32
    mult = mybir.AluOpType.mult
    add = mybir.AluOpType.add
    sub = mybir.AluOpType.subtract
    mx = mybir.AluOpType.max

    n_ad, n_ag, n_ac, n_dd = splits or SPLITS
    assert n_ad + n_ag + n_ac + n_dd == K

    nc = bacc.Bacc(None, target_bir_lowering=False)
    x_in = nc.declare_dram_parameter("xin", [P, F + 2 * K], f32, isOutput=False)
    out_d = nc.declare_dram_parameter("out", [P, F], f32, isOutput=True)

    # interleaved emission order: spread lanes so every engine has early work
    counters = {"AD": 0, "AG": n_ad, "AC": n_ad + n_ag, "DD": n_ad + n_ag + n_ac}
    seq = []
    remaining = {"AD": n_ad, "AG": n_ag, "AC": n_ac, "DD": n_dd}
    total = K
    while total > 0:
        for lane in ("AD", "DD", "AC", "AG"):
            if remaining[lane] > 0:
                seq.append((lane, counters[lane]))
                counters[lane] += 1
                remaining[lane] -= 1
                total -= 1

    with TileContext(nc) as tc:
        with (
            tc.tile_pool(name="io", bufs=1) as io_pool,
            tc.tile_pool(name="psum", bufs=1, space="PSUM") as psum_pool,
            tc.tile_pool(name="rp", bufs=4) as rp,
        ):
            xin_t = io_pool.tile([P, F + 2 * K], f32)
            xt = xin_t[:, :F]
            nbt = xin_t[:, F:F + K]          # -b_k columns
            sbt = xin_t[:, F + K:]           # -|w_k|*b_k columns
            acc_d = psum_pool.tile([P, F], f32)
            acc_gp = io_pool.tile([P, F], f32)
            acc_gn = io_pool.tile([P, F], f32)
            acc_c = [
                io_pool.tile([P, F], f32, name=f"acc_c{i}") for i in range(4)
            ]  # CCE accumulators: [pos0, pos1, neg0, neg1]
            res = io_pool.tile([P, F], f32)

            # Pre-warm the ACT table set (Relu) while the input DMA is in
            # flight: the table load (~2.7us) is data-independent.
            warm = io_pool.tile([P, 1], f32, name="warm")
            nc.vector.memset(warm[:], 0.0)
            nc.scalar.activation(
                warm[:], warm[:], mybir.ActivationFunctionType.Relu,
                bias=0.0, scale=1.0,
            )

            nc.sync.dma_start(out=xin_t[:], in_=x_in[:])

            for _ in range(repeat):
                first_d, first_gp, first_gn = [True], [True], [True]
                used_c = [False] * 4
                if n_ac:
                    for t in acc_c:
                        nc.gpsimd.memset(t[:], 0.0)
                ccnt = [0]

                for lane, k in seq:
                    if lane == "AD":
                        r = rp.tile([P, F], f32, name="r_ad", tag="r_ad")
                        nc.scalar.activation(
                            r[:], xt, mybir.ActivationFunctionType.Relu,
                            bias=nbt[:, k:k + 1], scale=1.0,
                        )
                        if first_d[0]:
                            nc.vector.tensor_scalar_mul(acc_d[:], r[:], float(w[k]))
                            first_d[0] = False
                        else:
                            nc.vector.scalar_tensor_tensor(
                                out=acc_d[:], in0=r[:], scalar=float(w[k]),
                                in1=acc_d[:], op0=mult, op1=add,
                            )
                    elif lane == "AG":
                        # r' = |w_k| * relu(x - b_k), sign handled by accumulator
                        r = rp.tile([P, F], f32, name="r_ag", tag="r_ag")
                        nc.scalar.activation(
                            r[:], xt, mybir.ActivationFunctionType.Relu,
                            bias=sbt[:, k:k + 1], scale=float(abs(w[k])),
                        )
                        acc_g, flag = (
                            (acc_gp, first_gp) if w[k] >= 0 else (acc_gn, first_gn)
                        )
                        if flag[0]:
                            nc.gpsimd.tensor_copy(out=acc_g[:], in_=r[:])
                            flag[0] = False
                        else:
                            nc.gpsimd.tensor_tensor(
                                out=acc_g[:], in0=acc_g[:], in1=r[:], op=add,
                            )
                    elif lane == "AC":
                        # r' = |w_k| * relu(x - b_k) accumulated by compute-DMA
                        r = rp.tile([P, F], f32, name="r_ac", tag="r_ac")
                        nc.scalar.activation(
                            r[:], xt, mybir.ActivationFunctionType.Relu,
                            bias=sbt[:, k:k + 1], scale=float(abs(w[k])),
                        )
                        par = ccnt[0] % 2
                        ccnt[0] += 1
                        ci = par if w[k] >= 0 else 2 + par
                        used_c[ci] = True
                        nc.gpsimd.dma_start(
                            out=acc_c[ci][:], in_=r[:], accum_op=add,
                        )
                    else:  # DD: DVE ts-relu + DVE stt mac
                        r = rp.tile([P, F], f32, name="r_dd", tag="r_dd")
                        nc.vector.tensor_scalar(
                            r[:], xt, float(b[k]), 0.0, sub, mx,
                        )
                        nc.vector.scalar_tensor_tensor(
                            out=acc_d[:], in0=r[:], scalar=float(w[k]),
                            in1=acc_d[:], op0=mult, op1=add,
                        )

                # GpSimd pre-combines its accumulator pairs while DVE finishes
                # the mac chain, shortening the serial DVE tail.
                gp_used, gn_used = not first_gp[0], not first_gn[0]
                if gp_used and gn_used:
                    nc.gpsimd.tensor_tensor(
                        out=acc_gp[:], in0=acc_gp[:], in1=acc_gn[:], op=sub,
                    )
                    gn_used = False
                if used_c[0] and used_c[1]:
                    nc.gpsimd.tensor_tensor(
                        out=acc_c[0][:], in0=acc_c[0][:], in1=acc_c[1][:], op=add,
                    )
                    used_c[1] = False
                if used_c[2] and used_c[3]:
                    nc.gpsimd.tensor_tensor(
                        out=acc_c[2][:], in0=acc_c[2][:], in1=acc_c[3][:], op=add,
                    )
                    used_c[3] = False

                # res = acc_d + acc_gp(-acc_gn) + cce_pos - cce_neg  (DVE)
                cur = acc_d
                terms = []
                if gp_used:
                    terms.append((acc_gp, add))
                if gn_used:
                    terms.append((acc_gn, sub))
                for i, t in enumerate(acc_c):
                    if used_c[i]:
                        terms.append((t, add if i < 2 else sub))
                # finals + output DMA in column halves so the first half's
                # store overlaps the second half's combines
                H = F // 2
                for c0, c1 in ((0, H), (H, F)):
                    curh = cur
                    for t, op in terms:
                        nc.vector.tensor_tensor(
                            out=res[:, c0:c1], in0=curh[:, c0:c1],
                            in1=t[:, c0:c1], op=op,
                        )
                        curh = res
                    if curh is not res:
                        nc.vector.tensor_copy(out=res[:, c0:c1], in_=curh[:, c0:c1])
                    nc.sync.dma_start(out=out_d[:, c0:c1], in_=res[:, c0:c1])
    return nc


def _prep_inputs(x, x_list, y_list):
    w, b = _tables(np.asarray(x_list), np.asarray(y_list))
    x_flat = np.ascontiguousarray(np.asarray(x, dtype=np.float32).reshape(-1))
    assert x_flat.size == M_TOTAL, x_flat.size
    padded = np.zeros(N_CORES * PER_CORE, np.float32)
    padded[:M_TOTAL] = x_flat
    shards = padded.reshape(N_CORES, P, F)
    nb_tile = np.broadcast_to((-b).reshape(1, K), (P, K)).astype(np.float32)
    sb = (-(np.abs(w.astype(np.float64)) * b.astype(np.float64))).astype(np.float32)
    sb_tile = np.broadcast_to(sb.reshape(1, K), (P, K)).astype(np.float32)
    in_maps = []
    for i in range(N_CORES):
        xin = np.concatenate([shards[i], nb_tile, sb_tile], axis=1)
        in_maps.append({"xin": np.ascontiguousarray(xin)})
    return w, b, in_maps


def run(x, x_list, y_list, trace=False, repeat=1, **spmd_kwargs):
    from concourse.bass_utils import run_bass_kernel_spmd

    w, b, in_maps = _prep_inputs(x, x_list, y_list)
    nc = _build_graph(w, b, repeat=repeat)
    if not nc.is_finalized():
        nc.finalize()
    res = run_bass_kernel_spmd(
        nc, in_maps, core_ids=list(range(N_CORES)), trace=trace, **spmd_kwargs
    )
    outs = np.stack([res.results[i]["out"] for i in range(N_CORES)])
    full = outs.reshape(-1)[:M_TOTAL].reshape(M_TOTAL, 1).astype(np.float32)
    return full, res


def kernel(x, x_list, y_list):
    full, _ = run(x, x_list, y_list, trace=False)
    return full
